# revision 26
# baseline (speedup 1.0000x reference)
"""Distributed Trainium2 kernel for nn_CEMA_34445637714419.

Math (from the reference):
    scale[d] = sum_{j,k} eta[d,j] * cos(j*omega[k]*2pi/h) * alpha[d,k] * beta[d,k]
    y[b,d]   = x[b,d] * scale[d]

The (d,) scale vector costs ~17 MFLOP — computed on host in float64.
The device kernel is the pure memory-bound part: stream x (16384,2048) f32
through SBUF, multiply by the partition-replicated scale row, stream out.
Sharding: x split along batch across 8 NeuronCores (data parallel),
scale replicated.

Measured HW model (trn2, this kernel):
  - Two HWDGE rings (SP, ACT), each ~212 GB/s, ~4 outstanding DMAs deep.
  - Mixed directions ACROSS rings sustain ~425 GB/s combined (fabric cap);
    mixing directions WITHIN a ring collapses to ~350.
  - Per-direction HBM cap ~340 GB/s; SWDGE ring ~208 GB/s, slow spin-up.
So: reads stream on SP, writes on ACT, equal bytes per ring; the scale is
broadcast on-chip (K=1 PE matmul against ones) from an 8 KiB read instead
of burning a 1 MiB replicated read; the last tile is tapered so the final
read->mul->write dependency chain is short.
"""

import math
from contextlib import ExitStack

import numpy as np

try:
    import concourse.bass as bass
except ImportError:  # grading container may not have it on sys.path yet
    import sys

    sys.path.insert(0, "/opt/trn_rl_repo")
    import concourse.bass as bass

import concourse.bacc as bacc
import concourse.mybir as mybir
from concourse.bass_utils import run_bass_kernel_spmd
from concourse.tile import TileContext

BATCH = 16384
D = 2048
H = 64
N_CORES = 8
SHARD = BATCH // N_CORES  # 2048 rows per core
P = 128  # SBUF partitions
N_TILES = SHARD // P  # 16 tiles of (128, 2048) = 1 MiB each


def build_nc() -> bacc.Bacc:
    nc = bacc.Bacc(
        "TRN2", target_bir_lowering=False, debug=False, num_devices=N_CORES
    )
    f32 = mybir.dt.float32
    x_ext = nc.declare_dram_parameter("x", [SHARD, D], f32, isOutput=False)
    s_ext = nc.declare_dram_parameter("scale", [1, D], f32, isOutput=False)
    out_ext = nc.declare_dram_parameter("out", [SHARD, D], f32, isOutput=True)

    # Column taper of the last SP-read row-block: the final
    # read->mul->write chain shrinks from ~9 us to ~4 us.
    TAPER = [(0, 1024), (1024, 512), (1536, 256), (1792, 256)]
    TAPER_TILE = N_TILES - 3  # tile 13: last tile on the SP read ring
    SWDGE_TILES = (N_TILES - 2, N_TILES - 1)  # tiles 14,15 on the gpsimd ring

    with TileContext(nc) as tc, ExitStack() as ctx:
        with (
            tc.tile_pool(name="const", bufs=1) as cpool,
            tc.tile_pool(name="psum", bufs=1, space="PSUM") as ppool,
            # One slot per distinct tag: every x tile gets its own SBUF
            # slot (16 MiB total), so there is no slot reuse and no
            # WAR/WAW waits — the TT ISA slot only fits one sem wait.
            tc.tile_pool(name="io", bufs=1) as pool,
        ):
            s_row = cpool.tile([1, D], f32)
            ones_t = cpool.tile([1, P], f32)
            scratch = cpool.tile([P, 1], f32)
            ps = ppool.tile([P, D], f32)
            nc.sync.dma_start(s_row[:], s_ext[:])  # 8 KiB on the SP head
            nc.vector.memset(ones_t[:], 1.0)
            # Broadcast scale across partitions: ones(1,128).T @ s(1,512)
            # per 512-col PSUM bank. K=1, so values are exact. The muls
            # read the scale straight from PSUM — no SBUF round-trip.
            for c in range(0, D, 512):
                nc.tensor.matmul(
                    ps[:, c : c + 512],
                    ones_t[:],
                    s_row[:, c : c + 512],
                    start=True,
                    stop=True,
                )
            # Tiny DVE read of ps: absorbs the PE dependency so every
            # tensor_mul below needs only its own x-DMA wait.
            nc.vector.tensor_copy(out=scratch[:], in_=ps[:, 0:1])

            tiles = [
                pool.tile([P, D], f32, name=f"t{i}", tag=f"t{i}")
                for i in range(N_TILES)
            ]
            # SWDGE reads issue immediately (no waits) and land early,
            # while the write ring is still idle — shortens the SP stream.
            for i in SWDGE_TILES:
                nc.gpsimd.dma_start(tiles[i][:], x_ext[i * P : (i + 1) * P, :])
            for i in range(N_TILES):
                t = tiles[i]
                rows = x_ext[i * P : (i + 1) * P, :]
                if i in SWDGE_TILES:
                    nc.vector.tensor_mul(out=t[:], in0=t[:], in1=ps[:])
                elif i == TAPER_TILE:
                    for c0, w in TAPER:
                        nc.sync.dma_start(t[:, c0 : c0 + w], rows[:, c0 : c0 + w])
                        nc.vector.tensor_mul(
                            out=t[:, c0 : c0 + w],
                            in0=t[:, c0 : c0 + w],
                            in1=ps[:, c0 : c0 + w],
                        )
                else:
                    nc.sync.dma_start(t[:], rows)
                    nc.vector.tensor_mul(out=t[:], in0=t[:], in1=ps[:])
            # Write ring (ACT): tile order, except the tapered tile's
            # pieces go last — they are the shortest final chain.
            worder = [j for j in range(N_TILES) if j != TAPER_TILE] + [TAPER_TILE]
            for j in worder:
                orows = out_ext[j * P : (j + 1) * P, :]
                if j == TAPER_TILE:
                    for c0, w in TAPER:
                        nc.scalar.dma_start(
                            orows[:, c0 : c0 + w], tiles[j][:, c0 : c0 + w]
                        )
                else:
                    nc.scalar.dma_start(orows, tiles[j][:])
    nc.finalize()
    return nc


def host_scale(alpha, omega, beta, eta) -> np.ndarray:
    h = omega.shape[0]
    j = np.arange(h, dtype=np.float64)
    theta = j[:, None] * omega[None, :].astype(np.float64) * (2.0 * math.pi / h)
    ct = np.cos(theta)
    ab = alpha.astype(np.float64) * beta.astype(np.float64)
    scale = np.einsum("dj,jk,dk->d", eta.astype(np.float64), ct, ab)
    return scale.astype(np.float32)


def run(x, scale, trace=False, tmpdir=None):
    nc = build_nc()
    scale_b = np.ascontiguousarray(scale[None, :])
    in_maps = [
        {"x": np.ascontiguousarray(x[c * SHARD : (c + 1) * SHARD]), "scale": scale_b}
        for c in range(N_CORES)
    ]
    res = run_bass_kernel_spmd(
        nc, in_maps, core_ids=list(range(N_CORES)), trace=trace, tmpdir=tmpdir
    )
    out = np.concatenate([res.results[c]["out"] for c in range(N_CORES)], axis=0)
    return out, res


def kernel(x, alpha, delta, omega, beta, eta):
    x = np.asarray(x, dtype=np.float32)
    scale = host_scale(
        np.asarray(alpha), np.asarray(omega), np.asarray(beta), np.asarray(eta)
    )
    out, _ = run(x, scale)
    return out
